# revision 1
# baseline (speedup 1.0000x reference)
"""Trainium2 Bass kernel: additive (Bahdanau-style) attention readout.

Reference computation (per batch b):
    energy  = tanh(enc @ W1.T + dec_b @ W2.T + W_b)      # (S, H)
    scores  = energy @ V + V_b, masked                   # (S,)
    attn    = softmax(scores)                            # (S,)
    context = attn @ enc                                 # (D,)

Sharding: data-parallel over batch across 8 NeuronCores (4 batches/core),
small weights replicated.  Host prep: enc cast to fp16 (pre-transposed to
[d, s] so et tiles load contiguously), W1.T pre-transposed (fp16), the tiny
dec projection + bias folded to a per-(h,batch) bias, and the mask + V_b
folded to an additive score penalty.

Device dataflow per batch (cost-model span ~552 us/core, PE-bound):
  - et tiles [p=d, k, s] stream in per k-chunk on the SP HWDGE queue.
  - pass1: psum[h,s] = sum_k W1T-chunk.T @ et-chunk   (fp16 PE, N=512)
  - tanh+bias on ScalarE (psum -> sbuf fp16), scores matmul V.T @ energy
    on PE (fp16), + penalty row on DVE.
  - softmax on the [1, S] score row (DVE reduce + ScalarE exp w/ accum),
    then attn bounced to DRAM (ACT HWDGE queue, to keep SP streaming).
  - context pass2: batches 0..bpc-2 on DVE (mult) + ScalarE (accum-reduce)
    over the resident transposed tiles — fully hidden under the next
    batch's pass1; the LAST batch runs on the then-idle PE against a
    host-shipped natural-layout slice, shortening the kernel tail.
"""

import numpy as np
import ml_dtypes

import concourse.bass as bass
import concourse.tile as tile
from concourse import bacc, mybir
from concourse.bass_utils import run_bass_kernel_spmd

# Problem shapes (hardcoded per contract).
B, S, D, H = 32, 2048, 2048, 1024
NCORES = 8
BPC = B // NCORES  # batches per core

F32 = mybir.dt.float32
F32R = mybir.dt.float32r
BF16 = mybir.dt.bfloat16
F16 = mybir.dt.float16
AF = mybir.ActivationFunctionType
ALU = mybir.AluOpType


def build_program(bpc=BPC, s=S, d=D, h=H, nt=512, nhalf=2, host_t=False):
    """Build the per-core Bass program (SPMD; identical on all cores).

    host_t: if True, enc arrives pre-transposed from the host as
    [bpc, d, s] and et tiles load with plain DMAs (no xbar transpose).
    """
    P = 128
    KD = d // P            # enc-feature chunks (contraction of pass1)
    MH = h // P            # h chunks
    sh = s // nhalf        # tokens per s-half (et tile granularity)
    assert sh % nt == 0 and d % P == 0 and h % P == 0
    NTH = sh // nt         # token tiles per half

    nc = bacc.Bacc(None, target_bir_lowering=False)
    enc_shape = [bpc, d, s] if host_t else [bpc, s, d]
    enc = nc.declare_dram_parameter("enc", enc_shape, F16, isOutput=False)
    w1t = nc.declare_dram_parameter("w1t", [d, h], F16, isOutput=False)
    vt = nc.declare_dram_parameter("vt", [h], F16, isOutput=False)
    cbias = nc.declare_dram_parameter("cbias", [h, bpc], F32, isOutput=False)
    pen = nc.declare_dram_parameter("pen", [bpc, s], F32, isOutput=False)
    # natural-layout copy of the core's LAST batch, for the PE-based pass2
    # that shortens the kernel tail
    encn = nc.declare_dram_parameter("encn", [s, d], F16, isOutput=False)
    ctx_out = nc.declare_dram_parameter("ctx", [bpc, d], F32, isOutput=True)
    attn_dram = nc.dram_tensor("attn_bounce", [s], F32)

    with tile.TileContext(nc) as tc:
        with (
            tc.tile_pool(name="singles", bufs=1) as singles,
            tc.tile_pool(name="et_pool", bufs=3) as et_pool,
            tc.tile_pool(name="en_pool", bufs=3) as en_pool,
            tc.tile_pool(name="row_pool", bufs=2) as row_pool,
            tc.tile_pool(name="pen_pool", bufs=2) as pen_pool,
            tc.tile_pool(name="bc_pool", bufs=1) as bc_pool,
            tc.tile_pool(name="scr_pool", bufs=2) as scr_pool,
            tc.tile_pool(name="ctx_pool", bufs=2) as ctx_pool,
            tc.tile_pool(name="stat_pool", bufs=4) as stat_pool,
            tc.tile_pool(name="psum_mm", bufs=2, space="PSUM") as psum_mm,
            tc.tile_pool(name="psum_sc", bufs=2, space="PSUM") as psum_sc,
            tc.tile_pool(name="psum_ctx", bufs=1, space="PSUM") as psum_ctx,
        ):
            # Resident constants.  w1 is loaded per k-chunk on the gpsimd
            # queue so the SP queue can start streaming et immediately and
            # the first matmuls only wait for their own chunks.
            w1_sb = singles.tile([P, KD, h], F16)
            w1_r = w1t.rearrange("(ko p) h -> p ko h", p=P)
            w1ch = min(4, KD)
            for k in range(0, KD, w1ch):
                nc.gpsimd.dma_start(
                    w1_sb[:, k:k + w1ch, :], w1_r[:, k:k + w1ch, :]
                )
            vt_sb = singles.tile([P, MH], F16)
            nc.gpsimd.dma_start(vt_sb, vt.rearrange("(m p) -> p m", p=P))
            cb_sb = singles.tile([P, MH, bpc], F32)
            nc.gpsimd.dma_start(cb_sb, cbias.rearrange("(m p) b -> p m b", p=P))

            for b in range(bpc):
                pen_row = pen_pool.tile([1, s], F32, tag="pen")
                nc.sync.dma_start(pen_row, pen[b][None, :])

                row = row_pool.tile([1, s], F32, tag="row")
                ets = []
                for hf in range(nhalf):
                    # Transposed enc tiles for this s-half:
                    # et[p, k, t] = enc[b, hf*sh + t, k*P + p]
                    # Loaded in k-chunks so pass1's k-loop can start before
                    # the whole half has landed.
                    et = et_pool.tile([P, KD, sh], F16, tag="et")
                    KCH = min(4, KD)
                    for kc in range(0, KD, KCH):
                        if host_t:
                            nc.sync.dma_start(
                                et[:, kc:kc + KCH, :],
                                enc[
                                    b, kc * P:(kc + KCH) * P,
                                    hf * sh:(hf + 1) * sh,
                                ].rearrange("(ko p) t -> p ko t", p=P),
                            )
                        else:
                            for k in range(kc, kc + KCH):
                                nc.sync.dma_start_transpose(
                                    et[:, k, :],
                                    enc[
                                        b, hf * sh:(hf + 1) * sh,
                                        k * P:(k + 1) * P,
                                    ],
                                )
                    ets.append(et)
                    for n in range(NTH):
                        ng = hf * NTH + n  # global token-tile index
                        ps_sc = psum_sc.tile([1, nt], F32)
                        for m in range(MH):
                            ps = psum_mm.tile([P, nt], F32)
                            for k in range(KD):
                                nc.tensor.matmul(
                                    ps,
                                    w1_sb[:, k, m * P:(m + 1) * P],
                                    et[:, k, n * nt:(n + 1) * nt],
                                    start=(k == 0),
                                    stop=(k == KD - 1),
                                )
                            energy = en_pool.tile([P, nt], F16, tag="energy")
                            nc.scalar.activation(
                                energy, ps, AF.Tanh,
                                bias=cb_sb[:, m, b:b + 1], scale=1.0,
                            )
                            nc.tensor.matmul(
                                ps_sc,
                                vt_sb[:, m:m + 1],
                                energy,
                                start=(m == 0),
                                stop=(m == MH - 1),
                            )
                        # scores(+V_b, +mask penalty) into the batch row
                        nc.vector.tensor_tensor(
                            row[:, ng * nt:(ng + 1) * nt],
                            ps_sc,
                            pen_row[:, ng * nt:(ng + 1) * nt],
                            ALU.add,
                        )

                # Softmax over the full row (in place: row -> exp -> attn).
                negmax = stat_pool.tile([1, 1], F32, tag="negmax")
                nc.vector.tensor_reduce(
                    negmax, row, axis=mybir.AxisListType.X, op=ALU.max,
                    negate=True,
                )
                ssum = stat_pool.tile([1, 1], F32, tag="ssum")
                nc.scalar.activation(
                    row, row, AF.Exp, bias=negmax, scale=1.0, accum_out=ssum,
                )
                rinv = stat_pool.tile([1, 1], F32, tag="rinv")
                nc.vector.reciprocal(rinv, ssum)
                nc.vector.tensor_scalar_mul(row, row, rinv)

                # attn bounce to DRAM, on the ACT HWDGE queue so the SP
                # queue stays a pure stream of et loads.
                nc.scalar.dma_start(attn_dram[None, :], row)

                if b == bpc - 1:
                    # Last batch: pass2 on the (otherwise idle) PE using the
                    # natural-layout copy.  attn read back partition-major,
                    # cast fp32 -> fp16 during the SWDGE DMA.
                    SK = s // P
                    attn_part = stat_pool.tile([P, SK], F16, tag="attn_part")
                    nc.gpsimd.dma_start(
                        attn_part,
                        attn_dram[:].rearrange("(sk p) -> p sk", p=P),
                    )
                    ctx_ps = psum_ctx.tile([1, d], F32)
                    # at nt=512 each slice is exactly one 2KB zero region;
                    # only smaller (test) shapes need the check skipped
                    skipg = nt * 4 < 2048
                    NJ = min(4, SK)
                    skg = SK // NJ
                    for j in range(NJ):
                        ent = et_pool.tile([P, skg, d], F16, tag="et")
                        nc.sync.dma_start(
                            ent,
                            encn[j * skg * P:(j + 1) * skg * P, :].rearrange(
                                "(c p) dd -> p c dd", p=P
                            ),
                        )
                        for c in range(skg):
                            sk = j * skg + c
                            for dt_ in range(d // nt):
                                nc.tensor.matmul(
                                    ctx_ps[:, dt_ * nt:(dt_ + 1) * nt],
                                    attn_part[:, sk:sk + 1],
                                    ent[:, c, dt_ * nt:(dt_ + 1) * nt],
                                    start=(sk == 0),
                                    stop=(sk == SK - 1),
                                    skip_group_check=skipg,
                                )
                    ctx_row = ctx_pool.tile([1, d], F32, tag="ctxrow")
                    nc.vector.tensor_copy(ctx_row, ctx_ps)
                    nc.scalar.dma_start(ctx_out[b][None, :], ctx_row)
                else:
                    # Broadcast attn across partitions via a replicated
                    # (partition-step-0) SWDGE read.
                    attn_bc = bc_pool.tile([P, s], F32, tag="attn_bc")
                    attn_src = attn_dram[None, :]
                    attn_src = bass.AP(
                        tensor=attn_src.tensor,
                        offset=attn_src.offset,
                        ap=[[0, P]] + list(attn_src.ap[1:]),
                    )
                    nc.gpsimd.dma_start(attn_bc, attn_src)

                    # Pass 2: context[d] = sum_s attn[s] * enc[s, d] on DVE,
                    # reusing the resident transposed tiles.
                    # (TensorTensorReduce is not supported by this runtime,
                    # so multiply + reduce.)  hf outer so each half tile is
                    # fully consumed (and its pool slot released for batch
                    # b+1) as early as possible.
                    ctx_sb = ctx_pool.tile([P, KD], F32, tag="ctx")
                    for hf in range(nhalf):
                        for k in range(KD):
                            scratch = scr_pool.tile(
                                [P, sh], F32, tag="scratch"
                            )
                            nc.vector.tensor_tensor(
                                scratch,
                                ets[hf][:, k, :],
                                attn_bc[:, hf * sh:(hf + 1) * sh],
                                ALU.mult,
                            )
                            part = stat_pool.tile([P, 1], F32, tag="part")
                            # reduce on ScalarE (in-place copy + accumulator)
                            # so DVE only does the multiplies.
                            nc.scalar.activation(
                                scratch, scratch, AF.Copy, scale=1.0,
                                accum_out=part,
                            )
                            if hf == 0:
                                nc.vector.tensor_copy(
                                    ctx_sb[:, k:k + 1], part
                                )
                            else:
                                nc.vector.tensor_tensor(
                                    ctx_sb[:, k:k + 1], ctx_sb[:, k:k + 1],
                                    part, ALU.add,
                                )
                    nc.scalar.dma_start(
                        ctx_out[b].rearrange("(k p) -> p k", p=P), ctx_sb,
                    )
    nc.finalize()
    return nc


_PROGRAM_CACHE = {}
HOST_TRANSPOSE = True


def _get_program(key, **kwargs):
    if key not in _PROGRAM_CACHE:
        _PROGRAM_CACHE[key] = build_program(**kwargs)
    return _PROGRAM_CACHE[key]


def prep_inputs(enc_output, enc_mask, dec_hidden, W_w, W_b, V_w, V_b):
    """Host-side shard + prep: returns per-core in_maps."""
    enc = np.asarray(enc_output, dtype=np.float32)
    mask = np.asarray(enc_mask, dtype=np.float32)[..., 0]          # (B, S)
    dec = np.asarray(dec_hidden, dtype=np.float32)[0]              # (B, H)
    W = np.asarray(W_w, dtype=np.float32)                          # (H, 3H)
    Wb = np.asarray(W_b, dtype=np.float32)                         # (H,)
    V = np.asarray(V_w, dtype=np.float32)[0]                       # (H,)
    Vb = float(np.asarray(V_b, dtype=np.float32)[0])

    enc_nat = enc.astype(np.float16)  # (B, S, D)
    if HOST_TRANSPOSE:
        enc_bf = np.ascontiguousarray(enc_nat.transpose(0, 2, 1))  # (B, D, S)
    else:
        enc_bf = enc_nat
    w1t = np.ascontiguousarray(W[:, :D].T).astype(np.float16)  # (D, H)
    # Tiny dec projection folded into a per-(h, b) bias (0.01% of FLOPs).
    cbias_all = (dec @ W[:, D:].T + Wb).astype(np.float32)         # (B, H)
    pen_all = (np.where(mask > 0, 0.0, -1e30) + Vb).astype(np.float32)  # (B, S)

    in_maps = []
    for c in range(NCORES):
        sl = slice(c * BPC, (c + 1) * BPC)
        in_maps.append({
            "enc": enc_bf[sl],
            "w1t": w1t,
            "vt": V.astype(np.float16),
            "cbias": np.ascontiguousarray(cbias_all[sl].T),        # (H, BPC)
            "pen": np.ascontiguousarray(pen_all[sl]),
            "encn": np.ascontiguousarray(enc_nat[c * BPC + BPC - 1]),
        })
    return in_maps


def kernel(**inputs) -> np.ndarray:
    in_maps = prep_inputs(**inputs)
    nc = _get_program(("full", HOST_TRANSPOSE), host_t=HOST_TRANSPOSE)
    res = run_bass_kernel_spmd(nc, in_maps, list(range(NCORES)))
    out = np.concatenate(
        [res.results[c]["ctx"] for c in range(NCORES)], axis=0
    )
    return np.ascontiguousarray(out.astype(np.float32))


if __name__ == "__main__":
    rng = np.random.default_rng(0)
    inputs = {
        "enc_output": rng.standard_normal((B, S, D), dtype=np.float32),
        "enc_mask": np.ones((B, S, 1), dtype=np.float32),
        "dec_hidden": rng.standard_normal((1, B, H), dtype=np.float32),
        "W_w": (rng.standard_normal((H, 3 * H), dtype=np.float32)
                / np.sqrt(3 * H)),
        "W_b": np.zeros((H,), dtype=np.float32),
        "V_w": rng.standard_normal((1, H), dtype=np.float32) / np.sqrt(H),
        "V_b": np.zeros((1,), dtype=np.float32),
    }
    out = kernel(**inputs)
    print(out.shape, out.dtype, float(np.abs(out).mean()))

